# revision 31
# baseline (speedup 1.0000x reference)
"""Trainium2 Bass kernel for nn_BSplineKAN_44719199486017.

2-layer B-spline KAN on x[4, 4096, 512]. Data-parallel over 8 NeuronCores:
core c handles batch b=c//2, sequence half h=c%2 (2048 rows).

Math: the 4 cubic Cox-de Boor basis functions reduce exactly to
    N_j(u) = sum_k C[j,k] * relu(k-u)^3,    u = 517*(x-min)/(max-min)
so the spline matmul becomes 4 plane matmuls with host-folded weights
    wk[f, o] = +sum_j C[j,k] * sw[o, 4f+j]
on planes r_k^3 with r_k = relu(k-u), built via a relu chain from
r_4 = relu(-su*x + (4-sb)) (one fused scalar-engine activation).

Layer-0 min/max: each core also loads its pair partner's x shard and
reduces it locally — no collective on the critical path (a device
collective costs 15-30us end-to-end here). Layer-1 min/max (h1 lives
on-device only) uses one pair-group AllGather on a [128, 8] stat tile
(max, -min per feature-partition) + a local max fold; features stay on
partitions the whole way, so no DRAM rearranges are needed. A dummy
AllGather at kernel start absorbs the first-collective setup cost, and
the last chunk's h1 stats are reduced per row-group so the exchange
fires right after the final transpose.

h1 is stored fp16 as t = 32*(h1 + 0.279): silu's flat minimum (-0.2785)
means values near the per-feature min — the only region the spline
basis is sensitive to — sit near zero where fp16 is precise, and the
min/max normalization is affine-invariant so u is unchanged. This makes
the PE transposes, h1T copies and stat reduces all 16-bit.

Matmul planes and weights are fp16 (fast weight load + 2x DVE),
accumulation stays fp32 in PSUM; h1 and the min/max path stay fp32 (the
spline basis is sensitive to min/max precision). Scalar activations
stick to one table set (silu/relu/square) except one batched sqrt per
4-row-group chunk. LayerNorm+silu is fused into one PSUM-read
activation per group: silu(ps*rsig - mu*rsig). Plane building runs one
chunk ahead of the matmul/LN consumers on the vector queue so the PE
never waits on plane construction.
"""
import numpy as np
from contextlib import ExitStack

import concourse.bass as bass
import concourse.tile as tile
import concourse.mybir as mybir
from concourse import bacc
from concourse.bass_utils import run_bass_kernel_spmd

F32 = mybir.dt.float32
FP16 = mybir.dt.float16
AF = mybir.ActivationFunctionType
OP = mybir.AluOpType
AX = mybir.AxisListType

B, S, F = 4, 4096, 512
SH = S // 2          # rows per core
NFT = F // 128       # feature tiles (4)
PCW = 512            # plane-chunk width (rows per plane build)
NPC = SH // PCW      # plane chunks (4)
GPP = PCW // 128     # row groups per chunk (4)
N_CORES = 8
KNOT_SCALE = 517.0
EPS = 1e-5
PAIR_GROUPS = [[0, 1], [2, 3], [4, 5], [6, 7]]

BASIS_C = np.array([
    [1.0, 0.0, 0.0, 0.0],
    [-2.0, 0.25, 0.0, 0.0],
    [1.5, -0.75, 1.0 / 6.0, 0.0],
    [-2.0 / 3.0, 1.0, -2.0 / 3.0, 1.0 / 6.0],
], dtype=np.float64)  # [j, k-1]

_CACHE = {}


def _build(sim=False, fast_gb=True):
    nc = bacc.Bacc("TRN2", target_bir_lowering=False, debug=False,
                   num_devices=1 if sim else N_CORES)
    nc._sim_mode = sim

    xT = nc.declare_dram_parameter("xT", [F, SH], F32, isOutput=False)
    xP = nc.declare_dram_parameter("xP", [F, SH], F32, isOutput=False)
    W0 = nc.declare_dram_parameter("W0", [F, 5, F], FP16, isOutput=False)
    W1 = nc.declare_dram_parameter("W1", [F, 5, F], FP16, isOutput=False)
    GB0 = nc.declare_dram_parameter("GB0", [128, 2, F], F32, isOutput=False)
    GB1 = nc.declare_dram_parameter("GB1", [128, 2, F], F32, isOutput=False)
    EYE = nc.declare_dram_parameter("EYE", [128, 128], F32, isOutput=False)
    OUT = nc.declare_dram_parameter("out", [SH, F], F32, isOutput=True)

    with ExitStack() as ctx:
        tc = ctx.enter_context(tile.TileContext(nc))
        dram = ctx.enter_context(tc.tile_pool(name="dram", bufs=1, space="DRAM"))
        wpool = ctx.enter_context(tc.tile_pool(name="w", bufs=1))
        xpool = ctx.enter_context(tc.tile_pool(name="x", bufs=1))
        hpool = ctx.enter_context(tc.tile_pool(name="h", bufs=1))
        lpool = ctx.enter_context(tc.tile_pool(name="l", bufs=2))
        stat = ctx.enter_context(tc.tile_pool(name="st", bufs=1))
        rpool = ctx.enter_context(tc.tile_pool(name="r", bufs=2))
        psum = ctx.enter_context(tc.tile_pool(name="ps", bufs=6, space="PSUM"))
        pstr = ctx.enter_context(tc.tile_pool(name="pstr", bufs=2, space="PSUM"))

        # ---- input loads, all on the sync queue in priority order -------
        # own x -> eye+W0 (gates base matmuls) -> partner x -> W1
        xts = []
        for ft in range(NFT):
            t = xpool.tile([128, SH], F32, tag=f"x{ft}", name=f"x{ft}")
            nc.sync.dma_start(t[:], xT.rearrange("(ft p) s -> ft p s", p=128)[ft])
            xts.append(t)
        # collective warmup: first collective in a NEFF pays extra setup;
        # burn it early on a dummy buffer while the x loads stream
        if not getattr(nc, "_sim_mode", False):
            wu_in = dram.tile([128, 8], F32, tag="wu_in", name="wu_in")
            wu_out = dram.tile([2, 128, 8], F32, tag="wu_out", name="wu_out")
            nc.gpsimd.collective_compute(
                "AllGather", OP.bypass,
                ins=[wu_in.opt()], outs=[wu_out.opt()],
                replica_groups=PAIR_GROUPS,
            )

        # partner x loads issue right behind own x on the sync queue; the
        # reduces are enqueued later so own stats go first on vector
        pxs = []
        for ft in range(NFT):
            for c in range(2):
                px = xpool.tile([128, 1024], F32, tag="px", name="px", bufs=4)
                nc.sync.dma_start(
                    px[:],
                    xP.rearrange("(ft p) (c s) -> ft c p s", p=128, s=1024)[ft][c])
                pxs.append(px)
        eye = wpool.tile([128, 128], F32)
        nc.sync.dma_start(eye[:], EYE[:])

        def load_w(li, Wp):
            wl = []
            for ft in range(NFT):
                t = wpool.tile([128, 5, F], FP16, tag=f"w{li}{ft}",
                               name=f"w{li}{ft}")
                nc.sync.dma_start(t[:], Wp.rearrange("(ft p) c n -> ft p c n",
                                                     p=128)[ft])
                wl.append(t)
            return wl

        wts = [load_w(0, W0), None]

        # ---- layer-0 own stats first, then partner, on the vector queue --
        pk_own = stat.tile([128, 8], F32, tag="pk_own", name="pk_own")
        for ft in range(NFT):
            nc.vector.tensor_reduce(pk_own[:, ft:ft + 1], xts[ft][:],
                                    axis=AX.X, op=OP.max)
            nc.vector.tensor_reduce(pk_own[:, 4 + ft:5 + ft], xts[ft][:],
                                    axis=AX.X, op=OP.min, negate=True)
        pk_p = stat.tile([128, 16], F32, tag="pk_p", name="pk_p")
        for ft in range(NFT):
            for c in range(2):
                px = pxs[ft * 2 + c]
                nc.vector.tensor_reduce(pk_p[:, c * 8 + ft:c * 8 + ft + 1],
                                        px[:], axis=AX.X, op=OP.max)
                nc.vector.tensor_reduce(pk_p[:, c * 8 + 4 + ft:c * 8 + 5 + ft],
                                        px[:], axis=AX.X, op=OP.min,
                                        negate=True)

        wts[1] = load_w(1, W1)
        gbts = []
        if not fast_gb:
            for li, GBp in enumerate((GB0, GB1)):
                t = wpool.tile([128, 2, F], F32, tag=f"gb{li}", name=f"gb{li}")
                nc.sync.dma_start(t[:], GBp[:])
                gbts.append(t)

        def suchain(st_, layer):
            """st_ [128, 8] combined (max, -min) -> (nsu, fb) [128, 4]."""
            rng = stat.tile([128, 4], F32, tag=f"rng{layer}", name=f"rng{layer}")
            nc.vector.tensor_tensor(rng[:], st_[:, 0:4], st_[:, 4:8], op=OP.add)
            rcp = stat.tile([128, 4], F32, tag=f"rcp{layer}", name=f"rcp{layer}")
            nc.vector.reciprocal(rcp[:], rng[:])
            su = stat.tile([128, 4], F32, tag=f"su{layer}", name=f"su{layer}")
            nc.vector.tensor_scalar(su[:], rcp[:], KNOT_SCALE, None, op0=OP.mult)
            nsu = stat.tile([128, 4], F32, tag=f"nsu{layer}", name=f"nsu{layer}")
            nc.vector.tensor_scalar(nsu[:], su[:], -1.0, None, op0=OP.mult)
            sb = stat.tile([128, 4], F32, tag=f"sb{layer}", name=f"sb{layer}")
            nc.vector.tensor_tensor(sb[:], su[:], st_[:, 4:8], op=OP.mult)
            fb = stat.tile([128, 4], F32, tag=f"fb{layer}", name=f"fb{layer}")
            nc.vector.tensor_scalar(fb[:], sb[:], -1.0, 4.0,
                                    op0=OP.mult, op1=OP.add)
            return nsu, fb

        # layer-0: pair-combine locally (no collective)
        stA = stat.tile([128, 8], F32, tag="stA", name="stA")
        nc.vector.tensor_tensor(stA[:], pk_p[:, 0:8], pk_p[:, 8:16], op=OP.max)
        st0 = stat.tile([128, 8], F32, tag="st0", name="st0")
        nc.vector.tensor_tensor(st0[:], stA[:], pk_own[:], op=OP.max)
        nsu0, fb0 = suchain(st0, 0)

        h1T = [hpool.tile([128, SH], F32, tag=f"h1T{ft}", name=f"h1T{ft}")
               for ft in range(NFT)]
        pk1 = stat.tile([128, 16], F32, tag="pk1", name="pk1")

        # base planes silu(src), fp16. Layer 0 now; layer 1 incrementally.
        sils = [[None] * NFT for _ in range(2)]
        for ft in range(NFT):
            t = lpool.tile([128, SH], FP16, tag=f"sil0_{ft}",
                           name=f"sil0_{ft}", bufs=1)
            nc.scalar.activation(t[:], xts[ft][:], AF.Silu)
            sils[0][ft] = t
            sils[1][ft] = lpool.tile([128, SH], FP16, tag=f"sil1_{ft}",
                                     name=f"sil1_{ft}", bufs=1)

        pss = {}

        def open_group(li, g):
            ps = psum.tile([128, F], F32, tag="y", name="y")
            pss[(li, g)] = ps
            for ft in range(NFT):
                nc.tensor.matmul(ps[:],
                                 sils[li][ft][:, g * 128:(g + 1) * 128],
                                 wts[li][ft][:, 0, :],
                                 start=(ft == 0), stop=False)

        for g in range(6):
            open_group(0, g)

        stats = [(nsu0, fb0), None]
        pend = None  # delayed transpose work (layer 0)

        def do_transposes(g, hrow):
            for ft in range(NFT):
                pt = pstr.tile([128, 128], F32, tag="ptr", name="ptr")
                nc.tensor.transpose(
                    pt[:], hrow[:, ft * 128:(ft + 1) * 128], eye[:])
                if ft % 2 == 0:
                    nc.vector.tensor_copy(
                        h1T[ft][:, g * 128:(g + 1) * 128], pt[:])
                else:
                    nc.scalar.copy(
                        h1T[ft][:, g * 128:(g + 1) * 128], pt[:])

        def build_planes(li, pc):
            """Enqueue spline-plane construction for chunk pc of layer li."""
            src = xts if li == 0 else h1T
            nsu, fb = stats[li]
            psl = slice(pc * PCW, (pc + 1) * PCW)
            planes = [[None] * 5 for _ in range(NFT)]
            for ft in range(NFT):
                r4 = lpool.tile([128, PCW], FP16, tag="r4", name="r4",
                                bufs=2)
                nc.scalar.activation(r4[:], src[ft][:, psl], AF.Relu,
                                     bias=fb[:, ft:ft + 1],
                                     scale=nsu[:, ft:ft + 1])
                rks = {4: r4}
                for k in (3, 2, 1):
                    rk = lpool.tile([128, PCW], FP16, tag="rk", name="rk",
                                    bufs=4)
                    nc.vector.tensor_scalar(rk[:], r4[:], float(4 - k),
                                            0.0, op0=OP.subtract,
                                            op1=OP.max)
                    rks[k] = rk
                for k in (4, 3, 2, 1):
                    rk = rks[k]
                    qk = lpool.tile([128, PCW], FP16, tag="qk", name="qk",
                                    bufs=3)
                    # keep each k-chain on one engine: k=2 lives fully on
                    # gpsimd, the rest build q and l on the same engine as
                    # their consumer so no chain hops engines mid-stream
                    if k == 4:
                        nc.scalar.activation(qk[:], rk[:], AF.Square)
                    elif k == 2:
                        nc.gpsimd.tensor_tensor(qk[:], rk[:], rk[:],
                                                op=OP.mult)
                    else:
                        nc.vector.tensor_tensor(qk[:], rk[:], rk[:],
                                                op=OP.mult)
                    lk = lpool.tile([128, PCW], FP16, tag=f"lk{k}_{ft}",
                                    name=f"lk{k}_{ft}", bufs=2)
                    leng = nc.gpsimd if k == 2 else nc.vector
                    leng.tensor_tensor(lk[:], qk[:], rk[:], op=OP.mult)
                    planes[ft][k] = lk
            return planes

        for li in range(2):
            wt = wts[li]

            for pc in range(NPC):
                planes = build_planes(li, pc)

                # ---- row-group matmuls for this chunk --------------------
                stp = stat.tile([128, 2, GPP], F32, tag="stp", name="stp",
                                bufs=2)
                gps = []
                for gg in range(GPP):
                    g = pc * GPP + gg
                    if (li, g) not in pss:
                        open_group(li, g)
                    ps = pss.pop((li, g))
                    gps.append(ps)
                    n_sp = 4 * NFT
                    i_sp = 0
                    for k in (1, 2, 3, 4):
                        for ft in range(NFT):
                            i_sp += 1
                            nc.tensor.matmul(
                                ps[:],
                                planes[ft][k][:, gg * 128:(gg + 1) * 128],
                                wt[ft][:, k, :],
                                start=False, stop=(i_sp == n_sp))

                # ---- batched LN + silu epilogue --------------------------
                # (last chunk of layer 1: two half-batches so the tail
                # starts draining before the final groups' matmuls end)
                halves = 2 if (li == 1 and pc == NPC - 1) else 1
                rsp = stat.tile([128, GPP], F32, tag="rsp", name="rsp",
                                bufs=2)
                nmr = stat.tile([128, GPP], F32, tag="nmr", name="nmr",
                                bufs=2)
                for hb in range(halves):
                    glo = hb * (GPP // halves)
                    ghi = (hb + 1) * (GPP // halves)
                    gsl = slice(glo, ghi)
                    for gg in range(glo, ghi):
                        st6 = stat.tile([128, 6], F32, tag="st6", name="st6",
                                        bufs=2)
                        nc.vector.bn_stats(st6[:], gps[gg][:])
                        nc.vector.bn_aggr(stp[:, :, gg], st6[:])
                    vep = stat.tile([128, GPP // halves], F32, tag="vep",
                                    name="vep", bufs=2)
                    nc.vector.tensor_scalar(vep[:], stp[:, 1, gsl], EPS,
                                            None, op0=OP.add)
                    sdp = stat.tile([128, GPP // halves], F32, tag="sdp",
                                    name="sdp", bufs=2)
                    nc.scalar.activation(sdp[:], vep[:], AF.Sqrt)
                    nc.vector.reciprocal(rsp[:, gsl], sdp[:])
                    nc.vector.scalar_tensor_tensor(nmr[:, gsl],
                                                   stp[:, 0, gsl], -1.0,
                                                   rsp[:, gsl], op0=OP.mult,
                                                   op1=OP.mult)

                for gg in range(GPP):
                    g = pc * GPP + gg
                    ps = gps[gg]
                    if fast_gb:
                        row = rpool.tile([128, F], F32, tag="row",
                                         name="row", bufs=3)
                        nc.scalar.activation(row[:], ps[:], AF.Silu,
                                             bias=nmr[:, gg:gg + 1],
                                             scale=rsp[:, gg:gg + 1])
                    else:
                        t2 = rpool.tile([128, F], F32, tag="t2", name="t2",
                                        bufs=2)
                        nc.vector.tensor_scalar(t2[:], ps[:],
                                                stp[:, 0, gg:gg + 1],
                                                rsp[:, gg:gg + 1],
                                                op0=OP.subtract, op1=OP.mult)
                        nc.vector.tensor_tensor(t2[:], t2[:],
                                                gbts[li][:, 0, :], op=OP.mult)
                        nc.gpsimd.tensor_tensor(t2[:], t2[:],
                                                gbts[li][:, 1, :], op=OP.add)
                        row = rpool.tile([128, F], F32, tag="row",
                                         name="row", bufs=3)
                        nc.scalar.activation(row[:], t2[:], AF.Silu)

                    if li == 0:
                        if pend is not None:
                            do_transposes(*pend)
                        pend = (g, row)
                    else:
                        nc.sync.dma_start(OUT[:][g * 128:(g + 1) * 128, :],
                                          row[:])

                # layer-0 tail per completed 1024 columns of h1T:
                # flush transposes, fill layer-1 sil, tree+reduce L1 stats
                if li == 0 and pc % 2 == 1:
                    if pend is not None:
                        do_transposes(*pend)
                        pend = None
                    c = pc // 2
                    h0 = c * 1024
                    for ft in range(NFT):
                        nc.scalar.activation(sils[1][ft][:, h0:h0 + 1024],
                                             h1T[ft][:, h0:h0 + 1024],
                                             AF.Silu)
                        nc.vector.tensor_reduce(
                            pk1[:, c * 8 + ft:c * 8 + ft + 1],
                            h1T[ft][:, h0:h0 + 1024], axis=AX.X, op=OP.max)
                        nc.vector.tensor_reduce(
                            pk1[:, c * 8 + 4 + ft:c * 8 + 5 + ft],
                            h1T[ft][:, h0:h0 + 1024], axis=AX.X, op=OP.min,
                            negate=True)

            if li == 0:
                # layer-1 stats: pair AllGather via DRAM + local max fold
                st1 = stat.tile([128, 8], F32, tag="st1", name="st1")
                nc.vector.tensor_tensor(st1[:], pk1[:, 0:8], pk1[:, 8:16],
                                        op=OP.max)
                cc_in = dram.tile([128, 8], F32, tag="cc_in", name="cc_in")
                cc_out = dram.tile([2, 128, 8], F32, tag="cc_out",
                                   name="cc_out")
                nc.sync.dma_start(cc_in[:], st1[:])
                if getattr(nc, "_sim_mode", False):
                    for r_ in range(2):
                        nc.sync.dma_start(cc_out[:][r_], cc_in[:])
                else:
                    nc.gpsimd.collective_compute(
                        "AllGather", OP.bypass,
                        ins=[cc_in.opt()], outs=[cc_out.opt()],
                        replica_groups=PAIR_GROUPS,
                    )
                res2 = stat.tile([128, 2, 8], F32, tag="res2", name="res2")
                nc.sync.dma_start(
                    res2[:], cc_out[:].rearrange("r p c -> p r c"))
                res = stat.tile([128, 8], F32, tag="res", name="res")
                nc.vector.tensor_tensor(res[:], res2[:, 0, :], res2[:, 1, :],
                                        op=OP.max)
                stats[1] = suchain(res, 1)
                # overlap the exchange with layer-1 base matmuls
                for g in range(6):
                    open_group(1, g)

    nc.compile()
    return nc


def _prep_inputs(x, bw0, sw0, g0, b0, bw1, sw1, g1, b1):
    def fold(bw, sw):
        sw4 = np.asarray(sw, np.float64).reshape(F, F, 4)
        wk = np.einsum('ofj,jk->kfo', sw4, BASIS_C)           # [4, f_in, o]
        W = np.empty((F, 5, F), np.float32)
        W[:, 0, :] = np.asarray(bw, np.float32).T
        for k in range(4):
            W[:, k + 1, :] = wk[k].astype(np.float32)
        return W.astype(np.float16)

    def gbpack(g, b):
        GB = np.empty((128, 2, F), np.float32)
        GB[:, 0, :] = np.asarray(g, np.float32)[None, :]
        GB[:, 1, :] = np.asarray(b, np.float32)[None, :]
        return GB

    W0 = fold(bw0, sw0)
    W1 = fold(bw1, sw1)
    GB0 = gbpack(g0, b0)
    GB1 = gbpack(g1, b1)
    eye = np.eye(128, dtype=np.float32)

    xs = []
    for c in range(N_CORES):
        b_, h_ = divmod(c, 2)
        xs.append(np.ascontiguousarray(
            np.asarray(x, np.float32)[b_, h_ * SH:(h_ + 1) * SH, :].T))

    in_maps = []
    for c in range(N_CORES):
        in_maps.append(dict(xT=xs[c], xP=xs[c ^ 1], W0=W0, W1=W1,
                            GB0=GB0, GB1=GB1, EYE=eye))
    return in_maps


def kernel(x, bw0, sw0, g0, b0, bw1, sw1, g1, b1):
    fast = (np.all(np.asarray(g0) == 1) and np.all(np.asarray(g1) == 1)
            and np.all(np.asarray(b0) == 0) and np.all(np.asarray(b1) == 0))
    key = "nc_fast" if fast else "nc_gen"
    if key not in _CACHE:
        _CACHE[key] = _build(fast_gb=fast)
    nc = _CACHE[key]
    in_maps = _prep_inputs(x, bw0, sw0, g0, b0, bw1, sw1, g1, b1)
    res = run_bass_kernel_spmd(nc, in_maps, list(range(N_CORES)))
    out = np.empty((B, S, F), np.float32)
    for c in range(N_CORES):
        b_, h_ = divmod(c, 2)
        out[b_, h_ * SH:(h_ + 1) * SH, :] = res.results[c]["out"]
    return out


# revision 32
# speedup vs baseline: 1.0303x; 1.0303x over previous
"""Trainium2 Bass kernel for nn_BSplineKAN_44719199486017.

2-layer B-spline KAN on x[4, 4096, 512]. Data-parallel over 8 NeuronCores:
core c handles batch b=c//2, sequence half h=c%2 (2048 rows).

Math: the 4 cubic Cox-de Boor basis functions reduce exactly to
    N_j(u) = sum_k C[j,k] * relu(k-u)^3,    u = 517*(x-min)/(max-min)
so the spline matmul becomes 4 plane matmuls with host-folded weights
    wk[f, o] = +sum_j C[j,k] * sw[o, 4f+j]
on planes r_k^3 with r_k = relu(k-u), built via a relu chain from
r_4 = relu(-su*x + (4-sb)) (one fused scalar-engine activation).

Layer-0 min/max: each core also loads its pair partner's x shard and
reduces it locally — no collective at all (a device collective costs
~30us end-to-end here). Layer-1 min/max (h1 lives on-device only) uses
one pair-group AllReduce(max) on a [128, 8] stat tile; features stay on
partitions the whole way, so no DRAM rearranges are needed.

Matmul planes and weights are fp16 (fast weight load + 2x DVE),
accumulation stays fp32 in PSUM; h1 and the min/max path stay fp32 (the
spline basis is sensitive to min/max precision). Scalar activations
stick to one table set (silu/relu/square) except one batched sqrt per
4-row-group chunk. LayerNorm+silu is fused into one PSUM-read
activation per group: silu(ps*rsig - mu*rsig). Plane building runs one
chunk ahead of the matmul/LN consumers on the vector queue so the PE
never waits on plane construction.
"""
import numpy as np
from contextlib import ExitStack

import concourse.bass as bass
import concourse.tile as tile
import concourse.mybir as mybir
from concourse import bacc
from concourse.bass_utils import run_bass_kernel_spmd

F32 = mybir.dt.float32
FP16 = mybir.dt.float16
AF = mybir.ActivationFunctionType
OP = mybir.AluOpType
AX = mybir.AxisListType

B, S, F = 4, 4096, 512
SH = S // 2          # rows per core
NFT = F // 128       # feature tiles (4)
PCW = 512            # plane-chunk width (rows per plane build)
NPC = SH // PCW      # plane chunks (4)
GPP = PCW // 128     # row groups per chunk (4)
N_CORES = 8
KNOT_SCALE = 517.0
EPS = 1e-5
PAIR_GROUPS = [[0, 1], [2, 3], [4, 5], [6, 7]]

BASIS_C = np.array([
    [1.0, 0.0, 0.0, 0.0],
    [-2.0, 0.25, 0.0, 0.0],
    [1.5, -0.75, 1.0 / 6.0, 0.0],
    [-2.0 / 3.0, 1.0, -2.0 / 3.0, 1.0 / 6.0],
], dtype=np.float64)  # [j, k-1]

_CACHE = {}


def _build(sim=False, fast_gb=True):
    nc = bacc.Bacc("TRN2", target_bir_lowering=False, debug=False,
                   num_devices=1 if sim else N_CORES)
    nc._sim_mode = sim

    xT = nc.declare_dram_parameter("xT", [F, SH], F32, isOutput=False)
    xP = nc.declare_dram_parameter("xP", [F, SH], F32, isOutput=False)
    W0 = nc.declare_dram_parameter("W0", [F, 5, F], FP16, isOutput=False)
    W1 = nc.declare_dram_parameter("W1", [F, 5, F], FP16, isOutput=False)
    GB0 = nc.declare_dram_parameter("GB0", [128, 2, F], F32, isOutput=False)
    GB1 = nc.declare_dram_parameter("GB1", [128, 2, F], F32, isOutput=False)
    EYE = nc.declare_dram_parameter("EYE", [128, 128], F32, isOutput=False)
    OUT = nc.declare_dram_parameter("out", [SH, F], F32, isOutput=True)

    with ExitStack() as ctx:
        tc = ctx.enter_context(tile.TileContext(nc))
        dram = ctx.enter_context(tc.tile_pool(name="dram", bufs=1, space="DRAM"))
        wpool = ctx.enter_context(tc.tile_pool(name="w", bufs=1))
        xpool = ctx.enter_context(tc.tile_pool(name="x", bufs=1))
        hpool = ctx.enter_context(tc.tile_pool(name="h", bufs=1))
        lpool = ctx.enter_context(tc.tile_pool(name="l", bufs=2))
        stat = ctx.enter_context(tc.tile_pool(name="st", bufs=1))
        rpool = ctx.enter_context(tc.tile_pool(name="r", bufs=2))
        psum = ctx.enter_context(tc.tile_pool(name="ps", bufs=6, space="PSUM"))
        pstr = ctx.enter_context(tc.tile_pool(name="pstr", bufs=2, space="PSUM"))

        # ---- input loads, all on the sync queue in priority order -------
        # own x -> eye+W0 (gates base matmuls) -> partner x -> W1
        xts = []
        for ft in range(NFT):
            t = xpool.tile([128, SH], F32, tag=f"x{ft}", name=f"x{ft}")
            nc.sync.dma_start(t[:], xT.rearrange("(ft p) s -> ft p s", p=128)[ft])
            xts.append(t)
        # collective warmup: first collective in a NEFF pays extra setup;
        # burn it early on a dummy buffer while the x loads stream
        if not getattr(nc, "_sim_mode", False):
            wu_in = dram.tile([128, 8], F32, tag="wu_in", name="wu_in")
            wu_out = dram.tile([2, 128, 8], F32, tag="wu_out", name="wu_out")
            nc.gpsimd.collective_compute(
                "AllGather", OP.bypass,
                ins=[wu_in.opt()], outs=[wu_out.opt()],
                replica_groups=PAIR_GROUPS,
            )

        # partner x loads issue right behind own x on the sync queue; the
        # reduces are enqueued later so own stats go first on vector
        pxs = []
        for ft in range(NFT):
            for c in range(2):
                px = xpool.tile([128, 1024], F32, tag="px", name="px", bufs=4)
                nc.sync.dma_start(
                    px[:],
                    xP.rearrange("(ft p) (c s) -> ft c p s", p=128, s=1024)[ft][c])
                pxs.append(px)
        eye = wpool.tile([128, 128], F32)
        nc.sync.dma_start(eye[:], EYE[:])

        def load_w(li, Wp):
            wl = []
            for ft in range(NFT):
                t = wpool.tile([128, 5, F], FP16, tag=f"w{li}{ft}",
                               name=f"w{li}{ft}")
                nc.sync.dma_start(t[:], Wp.rearrange("(ft p) c n -> ft p c n",
                                                     p=128)[ft])
                wl.append(t)
            return wl

        wts = [load_w(0, W0), None]

        # ---- layer-0 own stats first, then partner, on the vector queue --
        pk_own = stat.tile([128, 8], F32, tag="pk_own", name="pk_own")
        for ft in range(NFT):
            nc.vector.tensor_reduce(pk_own[:, ft:ft + 1], xts[ft][:],
                                    axis=AX.X, op=OP.max)
            nc.vector.tensor_reduce(pk_own[:, 4 + ft:5 + ft], xts[ft][:],
                                    axis=AX.X, op=OP.min, negate=True)
        pk_p = stat.tile([128, 16], F32, tag="pk_p", name="pk_p")
        for ft in range(NFT):
            for c in range(2):
                px = pxs[ft * 2 + c]
                nc.vector.tensor_reduce(pk_p[:, c * 8 + ft:c * 8 + ft + 1],
                                        px[:], axis=AX.X, op=OP.max)
                nc.vector.tensor_reduce(pk_p[:, c * 8 + 4 + ft:c * 8 + 5 + ft],
                                        px[:], axis=AX.X, op=OP.min,
                                        negate=True)

        wts[1] = load_w(1, W1)
        gbts = []
        if not fast_gb:
            for li, GBp in enumerate((GB0, GB1)):
                t = wpool.tile([128, 2, F], F32, tag=f"gb{li}", name=f"gb{li}")
                nc.sync.dma_start(t[:], GBp[:])
                gbts.append(t)

        def suchain(st_, layer):
            """st_ [128, 8] combined (max, -min) -> (nsu, fb) [128, 4]."""
            rng = stat.tile([128, 4], F32, tag=f"rng{layer}", name=f"rng{layer}")
            nc.vector.tensor_tensor(rng[:], st_[:, 0:4], st_[:, 4:8], op=OP.add)
            rcp = stat.tile([128, 4], F32, tag=f"rcp{layer}", name=f"rcp{layer}")
            nc.vector.reciprocal(rcp[:], rng[:])
            su = stat.tile([128, 4], F32, tag=f"su{layer}", name=f"su{layer}")
            nc.vector.tensor_scalar(su[:], rcp[:], KNOT_SCALE, None, op0=OP.mult)
            nsu = stat.tile([128, 4], F32, tag=f"nsu{layer}", name=f"nsu{layer}")
            nc.vector.tensor_scalar(nsu[:], su[:], -1.0, None, op0=OP.mult)
            sb = stat.tile([128, 4], F32, tag=f"sb{layer}", name=f"sb{layer}")
            nc.vector.tensor_tensor(sb[:], su[:], st_[:, 4:8], op=OP.mult)
            fb = stat.tile([128, 4], F32, tag=f"fb{layer}", name=f"fb{layer}")
            nc.vector.tensor_scalar(fb[:], sb[:], -1.0, 4.0,
                                    op0=OP.mult, op1=OP.add)
            return nsu, fb

        # layer-0: pair-combine locally (no collective)
        stA = stat.tile([128, 8], F32, tag="stA", name="stA")
        nc.vector.tensor_tensor(stA[:], pk_p[:, 0:8], pk_p[:, 8:16], op=OP.max)
        st0 = stat.tile([128, 8], F32, tag="st0", name="st0")
        nc.vector.tensor_tensor(st0[:], stA[:], pk_own[:], op=OP.max)
        nsu0, fb0 = suchain(st0, 0)

        h1T = [hpool.tile([128, SH], F32, tag=f"h1T{ft}", name=f"h1T{ft}")
               for ft in range(NFT)]
        pk1 = stat.tile([128, 16], F32, tag="pk1", name="pk1")

        # base planes silu(src), fp16. Layer 0 now; layer 1 incrementally.
        sils = [[None] * NFT for _ in range(2)]
        for ft in range(NFT):
            t = lpool.tile([128, SH], FP16, tag=f"sil0_{ft}",
                           name=f"sil0_{ft}", bufs=1)
            nc.scalar.activation(t[:], xts[ft][:], AF.Silu)
            sils[0][ft] = t
            sils[1][ft] = lpool.tile([128, SH], FP16, tag=f"sil1_{ft}",
                                     name=f"sil1_{ft}", bufs=1)

        pss = {}

        def open_group(li, g):
            ps = psum.tile([128, F], F32, tag="y", name="y")
            pss[(li, g)] = ps
            for ft in range(NFT):
                nc.tensor.matmul(ps[:],
                                 sils[li][ft][:, g * 128:(g + 1) * 128],
                                 wts[li][ft][:, 0, :],
                                 start=(ft == 0), stop=False)

        for g in range(6):
            open_group(0, g)

        stats = [(nsu0, fb0), None]
        pend = None  # delayed transpose work (layer 0)

        def do_transposes(g, hrow):
            for ft in range(NFT):
                pt = pstr.tile([128, 128], F32, tag="ptr", name="ptr")
                nc.tensor.transpose(
                    pt[:], hrow[:, ft * 128:(ft + 1) * 128], eye[:])
                if ft % 2 == 0:
                    nc.vector.tensor_copy(
                        h1T[ft][:, g * 128:(g + 1) * 128], pt[:])
                else:
                    nc.scalar.copy(
                        h1T[ft][:, g * 128:(g + 1) * 128], pt[:])

        def build_planes(li, pc):
            """Enqueue spline-plane construction for chunk pc of layer li."""
            src = xts if li == 0 else h1T
            nsu, fb = stats[li]
            psl = slice(pc * PCW, (pc + 1) * PCW)
            planes = [[None] * 5 for _ in range(NFT)]
            for ft in range(NFT):
                r4 = lpool.tile([128, PCW], FP16, tag="r4", name="r4",
                                bufs=2)
                nc.scalar.activation(r4[:], src[ft][:, psl], AF.Relu,
                                     bias=fb[:, ft:ft + 1],
                                     scale=nsu[:, ft:ft + 1])
                rks = {4: r4}
                for k in (3, 2, 1):
                    rk = lpool.tile([128, PCW], FP16, tag="rk", name="rk",
                                    bufs=4)
                    nc.vector.tensor_scalar(rk[:], r4[:], float(4 - k),
                                            0.0, op0=OP.subtract,
                                            op1=OP.max)
                    rks[k] = rk
                for k in (4, 3, 2, 1):
                    rk = rks[k]
                    qk = lpool.tile([128, PCW], FP16, tag="qk", name="qk",
                                    bufs=3)
                    if k == 4:
                        nc.scalar.activation(qk[:], rk[:], AF.Square)
                    elif k == 3:
                        nc.vector.tensor_tensor(qk[:], rk[:], rk[:],
                                                op=OP.mult)
                    else:
                        nc.gpsimd.tensor_tensor(qk[:], rk[:], rk[:],
                                                op=OP.mult)
                    lk = lpool.tile([128, PCW], FP16, tag=f"lk{k}_{ft}",
                                    name=f"lk{k}_{ft}", bufs=2)
                    nc.vector.tensor_tensor(lk[:], qk[:], rk[:],
                                            op=OP.mult)
                    planes[ft][k] = lk
            return planes

        for li in range(2):
            wt = wts[li]

            for pc in range(NPC):
                planes = build_planes(li, pc)

                # ---- row-group matmuls for this chunk --------------------
                stp = stat.tile([128, 2, GPP], F32, tag="stp", name="stp",
                                bufs=2)
                gps = []
                for gg in range(GPP):
                    g = pc * GPP + gg
                    if (li, g) not in pss:
                        open_group(li, g)
                    ps = pss.pop((li, g))
                    gps.append(ps)
                    n_sp = 4 * NFT
                    i_sp = 0
                    for k in (1, 2, 3, 4):
                        for ft in range(NFT):
                            i_sp += 1
                            nc.tensor.matmul(
                                ps[:],
                                planes[ft][k][:, gg * 128:(gg + 1) * 128],
                                wt[ft][:, k, :],
                                start=False, stop=(i_sp == n_sp))

                # ---- batched LN + silu epilogue --------------------------
                # (last chunk of layer 1: two half-batches so the tail
                # starts draining before the final groups' matmuls end)
                halves = 2 if (li == 1 and pc == NPC - 1) else 1
                rsp = stat.tile([128, GPP], F32, tag="rsp", name="rsp",
                                bufs=2)
                nmr = stat.tile([128, GPP], F32, tag="nmr", name="nmr",
                                bufs=2)
                for hb in range(halves):
                    glo = hb * (GPP // halves)
                    ghi = (hb + 1) * (GPP // halves)
                    gsl = slice(glo, ghi)
                    for gg in range(glo, ghi):
                        st6 = stat.tile([128, 6], F32, tag="st6", name="st6",
                                        bufs=2)
                        nc.vector.bn_stats(st6[:], gps[gg][:])
                        nc.vector.bn_aggr(stp[:, :, gg], st6[:])
                    vep = stat.tile([128, GPP // halves], F32, tag="vep",
                                    name="vep", bufs=2)
                    nc.vector.tensor_scalar(vep[:], stp[:, 1, gsl], EPS,
                                            None, op0=OP.add)
                    sdp = stat.tile([128, GPP // halves], F32, tag="sdp",
                                    name="sdp", bufs=2)
                    nc.scalar.activation(sdp[:], vep[:], AF.Sqrt)
                    nc.vector.reciprocal(rsp[:, gsl], sdp[:])
                    nc.vector.scalar_tensor_tensor(nmr[:, gsl],
                                                   stp[:, 0, gsl], -1.0,
                                                   rsp[:, gsl], op0=OP.mult,
                                                   op1=OP.mult)

                for gg in range(GPP):
                    g = pc * GPP + gg
                    ps = gps[gg]
                    if fast_gb:
                        row = rpool.tile([128, F], F32, tag="row",
                                         name="row", bufs=3)
                        nc.scalar.activation(row[:], ps[:], AF.Silu,
                                             bias=nmr[:, gg:gg + 1],
                                             scale=rsp[:, gg:gg + 1])
                    else:
                        t2 = rpool.tile([128, F], F32, tag="t2", name="t2",
                                        bufs=2)
                        nc.vector.tensor_scalar(t2[:], ps[:],
                                                stp[:, 0, gg:gg + 1],
                                                rsp[:, gg:gg + 1],
                                                op0=OP.subtract, op1=OP.mult)
                        nc.vector.tensor_tensor(t2[:], t2[:],
                                                gbts[li][:, 0, :], op=OP.mult)
                        nc.gpsimd.tensor_tensor(t2[:], t2[:],
                                                gbts[li][:, 1, :], op=OP.add)
                        row = rpool.tile([128, F], F32, tag="row",
                                         name="row", bufs=3)
                        nc.scalar.activation(row[:], t2[:], AF.Silu)

                    if li == 0:
                        if pend is not None:
                            do_transposes(*pend)
                        pend = (g, row)
                    else:
                        nc.sync.dma_start(OUT[:][g * 128:(g + 1) * 128, :],
                                          row[:])

                # layer-0 tail per completed 1024 columns of h1T:
                # flush transposes, fill layer-1 sil, tree+reduce L1 stats
                if li == 0 and pc % 2 == 1:
                    if pend is not None:
                        do_transposes(*pend)
                        pend = None
                    c = pc // 2
                    h0 = c * 1024
                    for ft in range(NFT):
                        nc.scalar.activation(sils[1][ft][:, h0:h0 + 1024],
                                             h1T[ft][:, h0:h0 + 1024],
                                             AF.Silu)
                        nc.vector.tensor_reduce(
                            pk1[:, c * 8 + ft:c * 8 + ft + 1],
                            h1T[ft][:, h0:h0 + 1024], axis=AX.X, op=OP.max)
                        nc.vector.tensor_reduce(
                            pk1[:, c * 8 + 4 + ft:c * 8 + 5 + ft],
                            h1T[ft][:, h0:h0 + 1024], axis=AX.X, op=OP.min,
                            negate=True)

            if li == 0:
                # layer-1 stats: pair AllGather via DRAM + local max fold
                st1 = stat.tile([128, 8], F32, tag="st1", name="st1")
                nc.vector.tensor_tensor(st1[:], pk1[:, 0:8], pk1[:, 8:16],
                                        op=OP.max)
                cc_in = dram.tile([128, 8], F32, tag="cc_in", name="cc_in")
                cc_out = dram.tile([2, 128, 8], F32, tag="cc_out",
                                   name="cc_out")
                nc.sync.dma_start(cc_in[:], st1[:])
                if getattr(nc, "_sim_mode", False):
                    for r_ in range(2):
                        nc.sync.dma_start(cc_out[:][r_], cc_in[:])
                else:
                    nc.gpsimd.collective_compute(
                        "AllGather", OP.bypass,
                        ins=[cc_in.opt()], outs=[cc_out.opt()],
                        replica_groups=PAIR_GROUPS,
                    )
                res2 = stat.tile([128, 2, 8], F32, tag="res2", name="res2")
                nc.sync.dma_start(
                    res2[:], cc_out[:].rearrange("r p c -> p r c"))
                res = stat.tile([128, 8], F32, tag="res", name="res")
                nc.vector.tensor_tensor(res[:], res2[:, 0, :], res2[:, 1, :],
                                        op=OP.max)
                stats[1] = suchain(res, 1)
                # overlap the exchange with layer-1 base matmuls
                for g in range(6):
                    open_group(1, g)

    nc.compile()
    return nc


def _prep_inputs(x, bw0, sw0, g0, b0, bw1, sw1, g1, b1):
    def fold(bw, sw):
        sw4 = np.asarray(sw, np.float64).reshape(F, F, 4)
        wk = np.einsum('ofj,jk->kfo', sw4, BASIS_C)           # [4, f_in, o]
        W = np.empty((F, 5, F), np.float32)
        W[:, 0, :] = np.asarray(bw, np.float32).T
        for k in range(4):
            W[:, k + 1, :] = wk[k].astype(np.float32)
        return W.astype(np.float16)

    def gbpack(g, b):
        GB = np.empty((128, 2, F), np.float32)
        GB[:, 0, :] = np.asarray(g, np.float32)[None, :]
        GB[:, 1, :] = np.asarray(b, np.float32)[None, :]
        return GB

    W0 = fold(bw0, sw0)
    W1 = fold(bw1, sw1)
    GB0 = gbpack(g0, b0)
    GB1 = gbpack(g1, b1)
    eye = np.eye(128, dtype=np.float32)

    xs = []
    for c in range(N_CORES):
        b_, h_ = divmod(c, 2)
        xs.append(np.ascontiguousarray(
            np.asarray(x, np.float32)[b_, h_ * SH:(h_ + 1) * SH, :].T))

    in_maps = []
    for c in range(N_CORES):
        in_maps.append(dict(xT=xs[c], xP=xs[c ^ 1], W0=W0, W1=W1,
                            GB0=GB0, GB1=GB1, EYE=eye))
    return in_maps


def kernel(x, bw0, sw0, g0, b0, bw1, sw1, g1, b1):
    fast = (np.all(np.asarray(g0) == 1) and np.all(np.asarray(g1) == 1)
            and np.all(np.asarray(b0) == 0) and np.all(np.asarray(b1) == 0))
    key = "nc_fast" if fast else "nc_gen"
    if key not in _CACHE:
        _CACHE[key] = _build(fast_gb=fast)
    nc = _CACHE[key]
    in_maps = _prep_inputs(x, bw0, sw0, g0, b0, bw1, sw1, g1, b1)
    res = run_bass_kernel_spmd(nc, in_maps, list(range(N_CORES)))
    out = np.empty((B, S, F), np.float32)
    for c in range(N_CORES):
        b_, h_ = divmod(c, 2)
        out[b_, h_ * SH:(h_ + 1) * SH, :] = res.results[c]["out"]
    return out
